# revision 10
# baseline (speedup 1.0000x reference)
"""CKA (centered kernel alignment) on 8 Trainium2 NeuronCores — v2.

Math: with z = [x | y] (8192 x 3072) and C = z^T z, all three HSIC terms
reduce to Frobenius norms / bilinear forms of blocks of C:
    hsic(a,b) = ||Ca,b||^2 - (2/n) sa^T Ca,b sb + ||sa||^2 ||sb||^2 / n^2
where s = column sums of z.  C is symmetric, so only upper 128x512 blocks
are computed: diagonal 512x512 squares fully (weight 1), strictly-upper
blocks once (weight 2); Cxy blocks weight 1.  All weighting, the s-terms
and the final scalar are applied on the HOST from per-block partials.

Sharding: rows (n) split across 8 cores.  Each core computes all 84 real
block partials in fp8-e4m3 (inputs quantized on host; DoubleRow perf mode
= 2 fp8 MACs/cell/cycle), spills PSUM f32 -> fp8 at 1/64 scale into 4
DRAM group buffers (24/24/24/16 slots of 129 rows: 128 C rows + 1 aux row
carrying the core's local s-partial slice so the ReduceScatter itself
produces the globally-summed s slice per block).  The 4 ReduceScatters
(fp8, ~1.6MB each) pipeline against the block compute of later groups.
Post-RS stage2 per owned block: one fused square+reduce and one fused
(C * s_row)+reduce -> [128,22] partials; host assembles in float64.

Validated numerically (host sim incl. fp8 input quant + fp8 RDH
requantization at every reduce stage): rel err ~6e-3 vs exact (gate 2e-2);
with bf16 RS fallback ~3e-4.
"""

import numpy as np

N_CORES = 8
N = 8192
NS = N // N_CORES      # 1024 rows per core
D = 3072               # 2048 x-features | 1024 y-features
DX = 2048
P = 128
KT = NS // P           # 8 k-slices of 128 rows per core
NMT = D // P           # 24 m-tiles
NC4 = D // 512         # 6 512-col chunks
SCALE = 1.0 / 64.0
DESCALE2 = 64.0 * 64.0
SLOT_ROWS = 129        # 128 C rows + 1 aux (s) row

import os as _os
DOUBLE_ROW = _os.environ.get("K_DR", "1") == "1"   # fp8 DoubleRow perf mode
RS_FP8 = True          # ReduceScatter payload in fp8e4 (else bf16)

# ---- static block/slot tables (shared by device builder and host) ----
GROUP_M = [(0, 4), (4, 9), (9, 16), (16, 24)]
SLOTS = []             # slot -> (m, n4)
GROUP_START = []
for _m0, _m1 in GROUP_M:
    GROUP_START.append(len(SLOTS))
    for _m in range(_m0, _m1):
        for _n4 in range(_m // 4, NC4):
            SLOTS.append((_m, _n4))
N_REAL = len(SLOTS)                     # 84
GROUP_SIZE = [24, 24, 24, 16]           # last group has 4 pad slots
N_SLOTS = sum(GROUP_SIZE)               # 88
PER_CORE = [gs // N_CORES for gs in GROUP_SIZE]   # [3, 3, 3, 2]
JJ_BASE = [0, 3, 6, 9]
N_JJ = sum(PER_CORE)                    # 11 blocks per core post-RS


def _slot_group(s):
    for g in range(3, -1, -1):
        if s >= GROUP_START[g]:
            return g, s - GROUP_START[g]
    raise AssertionError


def _local_to_global(c, jj):
    g = min(jj // 3, 3)
    jl = jj - JJ_BASE[g]
    return GROUP_START[g] + PER_CORE[g] * c + jl


def _block_weight(m, n4):
    if m < 16 and n4 >= 4:
        return 1.0, "xy"
    if m < 16:
        return (1.0 if n4 == m // 4 else 2.0), "xx"
    return (1.0 if (n4 - 4) == (m - 16) // 4 else 2.0), "yy"


_COMPILED = None


def _build():
    import concourse.bacc as bacc
    import concourse.mybir as mybir
    import concourse.tile as tile

    f32 = mybir.dt.float32
    bf16 = mybir.dt.bfloat16
    fp8 = mybir.dt.float8e4
    rs_dt = fp8 if RS_FP8 else bf16

    nc = bacc.Bacc("TRN2", target_bir_lowering=False, debug=False,
                   num_devices=N_CORES)
    z = nc.dram_tensor("z", [NS, D], fp8, kind="ExternalInput")
    ind = nc.dram_tensor("ind", [NC4, N_SLOTS], bf16, kind="ExternalInput")
    out_part = nc.dram_tensor("part", [P, 2 * N_JJ], f32,
                              kind="ExternalOutput")
    out_svec = nc.dram_tensor("svec", [NC4, 512], f32, kind="ExternalOutput")

    rg = [list(range(N_CORES))]
    pm = mybir.MatmulPerfMode.DoubleRow if DOUBLE_ROW else None

    with tile.TileContext(nc) as tc:
        with (
            tc.tile_pool(name="persist", bufs=1) as persist,
            tc.tile_pool(name="spill", bufs=4) as spill,
            tc.tile_pool(name="dram", bufs=1, space="DRAM") as dram,
            tc.tile_pool(name="psum", bufs=8, space="PSUM") as psum,
        ):
            # ---------------- load z (fp8) ----------------
            zb = persist.tile([P, KT, D], fp8)
            for k in range(KT):
                nc.sync.dma_start(zb[:, k, :], z[k * P:(k + 1) * P, :])

            # DRAM group buffers for the ReduceScatters
            bufG = [dram.tile([GROUP_SIZE[g] * SLOT_ROWS, 512], rs_dt,
                              name=f"bufG{g}") for g in range(4)]
            chG = [dram.tile([GROUP_SIZE[g] * SLOT_ROWS // N_CORES, 512],
                             rs_dt, name=f"chG{g}") for g in range(4)]

            # ---------------- s-pass: column sums of z ----------------
            # tree-reduce the KT axis on DVE, then 6 ones-matmuls reduce
            # the 128 partitions; scatter the six [1,512] rows to s6f.
            t1 = persist.tile([P, 4, D], bf16)
            nc.vector.tensor_add(t1[:], zb[:, 0:4, :], zb[:, 4:8, :])
            t2 = persist.tile([P, 2, D], bf16)
            nc.vector.tensor_add(t2[:], t1[:, 0:2, :], t1[:, 2:4, :])
            t3 = persist.tile([P, 1, D], bf16)
            nc.vector.tensor_add(t3[:], t2[:, 0:1, :], t2[:, 1:2, :])

            onesP = persist.tile([P, 1], bf16)
            nc.vector.memset(onesP[:], 1.0)
            ones1 = persist.tile([1, P], rs_dt)
            nc.vector.memset(ones1[:], 1.0)

            s6f = persist.tile([NC4, 512], f32)
            stmp = [persist.tile([1, 512], f32, name=f"stmp{k}")
                    for k in range(NC4)]
            for k in range(NC4):
                ps = psum.tile([P, 512], f32, tag="ps", name="ps")
                nc.tensor.matmul(ps[0:1, :], onesP[:],
                                 t3[:, 0, k * 512:(k + 1) * 512],
                                 start=True, stop=True)
                nc.scalar.copy(stmp[k][:], ps[0:1, :])
                nc.sync.dma_start(s6f[k:k + 1, :], stmp[k][:])
            nc.sync.dma_start(out_svec[:], s6f[:])

            # aux rows: [N_SLOTS, 512] = ind^T @ (s6f * SCALE)
            s6b = persist.tile([NC4, 512], bf16)
            nc.vector.tensor_scalar_mul(s6b[:], s6f[:], SCALE)
            indb = persist.tile([NC4, N_SLOTS], bf16)
            nc.sync.dma_start(indb[:], ind[:])
            ps_aux = psum.tile([P, 512], f32, tag="ps", name="ps")
            nc.tensor.matmul(ps_aux[0:N_SLOTS, :], indb[:], s6b[:],
                             start=True, stop=True)
            aux_sb = persist.tile([N_SLOTS, 512], rs_dt)
            nc.vector.tensor_copy(aux_sb[:], ps_aux[0:N_SLOTS, :])
            for g in range(4):
                g0 = GROUP_START[g] if g < 3 else GROUP_START[3]
                bga = bufG[g][:].rearrange("(s r) w -> s r w", r=SLOT_ROWS)
                nc.sync.dma_start(
                    bga[:, 128, :],
                    aux_sb[g0:g0 + GROUP_SIZE[g], :])

            # zero the 4 pad slots' C rows (group 3, locals 12..15)
            zpad = persist.tile([P, 512], rs_dt)
            nc.vector.memset(zpad[:], 0.0)
            for lg in range(12, 16):
                nc.sync.dma_start(
                    bufG[3][lg * SLOT_ROWS:lg * SLOT_ROWS + P, :], zpad[:])

            # ---------------- main block loop ----------------
            def compute_m(m):
                dm = m // 4
                n4s = list(range(dm, NC4))
                pss = [psum.tile([P, 512], f32, tag="ps", name="ps")
                       for _ in n4s]
                if DOUBLE_ROW:
                    for j in range(KT // 2):
                        for i, n4 in enumerate(n4s):
                            nc.tensor.matmul(
                                pss[i][:],
                                zb[:, 2 * j:2 * j + 2, m * P:(m + 1) * P],
                                zb[:, 2 * j:2 * j + 2,
                                   n4 * 512:(n4 + 1) * 512],
                                start=(j == 0), stop=(j == KT // 2 - 1),
                                perf_mode=pm)
                else:
                    for j in range(KT):
                        for i, n4 in enumerate(n4s):
                            nc.tensor.matmul(
                                pss[i][:],
                                zb[:, j, m * P:(m + 1) * P],
                                zb[:, j, n4 * 512:(n4 + 1) * 512],
                                start=(j == 0), stop=(j == KT - 1))
                for i, n4 in enumerate(n4s):
                    slot = SLOTS.index((m, n4))
                    g, lg = _slot_group(slot)
                    st = spill.tile([P, 512], rs_dt, tag="st", name="st",
                                    bufs=32)
                    nc.vector.tensor_scalar_mul(st[:], pss[i][:], SCALE)
                    nc.sync.dma_start(
                        bufG[g][lg * SLOT_ROWS:lg * SLOT_ROWS + P, :],
                        st[:])

            import os
            use_rs = os.environ.get("K_NO_RS", "") != "1"
            for g in range(4):
                for m in range(*GROUP_M[g]):
                    compute_m(m)
                if use_rs:
                    nc.gpsimd.collective_compute(
                        "ReduceScatter", mybir.AluOpType.add,
                        replica_groups=rg,
                        ins=[bufG[g][:]], outs=[chG[g][:]])
                else:
                    nrows = GROUP_SIZE[g] * SLOT_ROWS // N_CORES
                    nc.sync.dma_start(chG[g][:], bufG[g][0:nrows, :])

            # ---------------- stage 2: per-chunk partials ----------------
            acc = persist.tile([P, 2 * N_JJ], f32)
            for g in range(4):
                for jl in range(PER_CORE[g]):
                    jj = JJ_BASE[g] + jl
                    ct = spill.tile([P, 512], rs_dt, tag="ct", name="ct",
                                    bufs=4)
                    nc.sync.dma_start(
                        ct[:],
                        chG[g][jl * SLOT_ROWS:jl * SLOT_ROWS + P, :])
                    at = spill.tile([1, 512], rs_dt, tag="at", name="at",
                                    bufs=4)
                    nc.sync.dma_start(
                        at[:],
                        chG[g][jl * SLOT_ROWS + P:jl * SLOT_ROWS + P + 1, :])
                    # broadcast aux row down 128 partitions via outer product
                    ps_o = psum.tile([P, 512], f32, tag="ps", name="ps")
                    nc.tensor.matmul(ps_o[:], ones1[:], at[:],
                                     start=True, stop=True)
                    scr = spill.tile([P, 512], f32, tag="scr", name="scr",
                                     bufs=4)
                    nc.vector.tensor_mul(scr[:], ct[:], ct[:])
                    nc.vector.tensor_reduce(
                        out=acc[:, jj:jj + 1], in_=scr[:],
                        axis=mybir.AxisListType.X, op=mybir.AluOpType.add)
                    scr2 = spill.tile([P, 512], f32, tag="scr", name="scr",
                                      bufs=4)
                    nc.vector.tensor_mul(scr2[:], ct[:], ps_o[:])
                    nc.vector.tensor_reduce(
                        out=acc[:, N_JJ + jj:N_JJ + jj + 1], in_=scr2[:],
                        axis=mybir.AxisListType.X, op=mybir.AluOpType.add)

            nc.sync.dma_start(out_part[:], acc[:])

    nc.compile()
    return nc


def _get_compiled():
    global _COMPILED
    if _COMPILED is None:
        _COMPILED = _build()
    return _COMPILED


def _make_ind():
    import ml_dtypes
    ind = np.zeros((NC4, N_SLOTS), dtype=ml_dtypes.bfloat16)
    for s, (_m, n4) in enumerate(SLOTS):
        ind[n4, s] = 1.0
    return ind


def _run(x, y, trace=False):
    import ml_dtypes
    from concourse import bass_utils
    nc = _get_compiled()
    x = np.ascontiguousarray(np.asarray(x, dtype=np.float32))
    y = np.ascontiguousarray(np.asarray(y, dtype=np.float32))
    zq = np.concatenate([x, y], axis=1).astype(ml_dtypes.float8_e4m3)
    ind = _make_ind()
    in_maps = [{"z": zq[r * NS:(r + 1) * NS], "ind": ind}
               for r in range(N_CORES)]
    res = bass_utils.run_bass_kernel_spmd(
        nc, in_maps, core_ids=list(range(N_CORES)), trace=trace)

    # ---------------- host assembly (float64) ----------------
    s = np.zeros(D, dtype=np.float64)
    for r in range(N_CORES):
        s += np.asarray(res.results[r]["svec"], dtype=np.float64).reshape(D)
    F2 = {"xx": 0.0, "xy": 0.0, "yy": 0.0}
    Q = {"xx": 0.0, "xy": 0.0, "yy": 0.0}
    for c in range(N_CORES):
        p = np.asarray(res.results[c]["part"], dtype=np.float64)
        for jj in range(N_JJ):
            slot = _local_to_global(c, jj)
            if slot >= N_REAL:
                continue
            m, n4 = SLOTS[slot]
            w, key = _block_weight(m, n4)
            F2[key] += w * p[:, jj].sum() * DESCALE2
            Q[key] += w * (s[m * P:(m + 1) * P] * p[:, N_JJ + jj]).sum() \
                * DESCALE2
    sx2 = float(s[:DX] @ s[:DX])
    sy2 = float(s[DX:] @ s[DX:])
    nn = float(N)
    hxx = F2["xx"] - 2.0 / nn * Q["xx"] + sx2 * sx2 / nn ** 2
    hxy = F2["xy"] - 2.0 / nn * Q["xy"] + sx2 * sy2 / nn ** 2
    hyy = F2["yy"] - 2.0 / nn * Q["yy"] + sy2 * sy2 / nn ** 2
    val = np.float32(hxy / (np.sqrt(hxx * hyy) + 1e-8))
    return np.asarray(val, dtype=np.float32), res


def kernel(x, y):
    val, _ = _run(x, y, trace=False)
    return val
